# revision 1
# baseline (speedup 1.0000x reference)
"""DAG propagation (layered GNN message passing) on 8 trn2 NeuronCores.

The reference reduces exactly (verified numerically) to:
    out[level 0] = X_0 @ W_r^T
    out[level l] = tanh(X_l @ W_r^T + (sum_k out[level l-1][child_k]) @ W_l^T + b_l)

Sharding: each core owns a contiguous 2048-node slice of every level.
Per level, each core dma_gathers its parents' 16384 children rows from a
replicated full h_prev (core-local DRAM), reduces the 8 children per
parent on the vector engine, computes its 2048 output rows, and an
AllGather replicates the new level for the next iteration.

The critical resource is the GpSimd Q7 SWDGE descriptor generation
(~8ns/gathered row, 114K rows/core, ~900us serial Q7 work — the floor of
this design). Gathers are chunked 4x per level so transfers/compute of
chunk c overlap descriptor generation of chunk c+1. (An experimental
prepare_only/trigger_dma mode that overlaps descgen with the AllGather
is behind DAGPROP_PREP=1; it is not stable on this runtime.)

Layouts: node n of a core's 2048-node shard lives at (partition, slot)
= (n // 16, n % 16) in node-major tiles (all HBM accesses per-partition
contiguous), and at column (n % 16)*128 + n // 16 in feature-major
[64, 2048] tiles, matching what PE transposes of slot-slices produce.
"""

import os
import numpy as np

import concourse.bass as bass
import concourse.bacc as bacc
import concourse.tile as tile
from concourse.tile import add_dep_helper
from concourse import mybir
from concourse.bass_utils import run_bass_kernel_spmd
from concourse.masks import make_identity

# Problem constants (hardcoded per spec; asserted against inputs at runtime).
L = 8            # levels
NPL = 16384      # nodes per level
CH = 8           # children per node
D = 64           # feature dim
M = 8            # cores
S = NPL // M     # 2048: shard nodes per level per core
G = S // 128     # 16: slots per partition in node-major tiles
NCHUNK = 4       # gather chunks per level
CIDX = S * CH // NCHUNK   # 4096 gathered rows per chunk
F32 = mybir.dt.float32
I16 = mybir.dt.int16
NPG = (S * CH) // 16      # 1024: idx columns per level (wrapped int16 layout)

LAST_STATS: dict = {}


def _nodemajor(ap):
    """View a [S, D] DRAM slice as [128, G*D] with row p*G+r on partition p."""
    return ap.rearrange("(p r) d -> p (r d)", p=128)


def _build_kernel():
    nc = bacc.Bacc(
        "TRN2",
        target_bir_lowering=False,
        debug=False,
        num_devices=M,
        dynamic_dma_scratch_size=32768,
    )

    x_in = nc.dram_tensor("x_shard", [L * S, D], F32, kind="ExternalInput")
    idx_in = nc.dram_tensor("idx", [128, (L - 1) * NPG], I16, kind="ExternalInput")
    wl_in = nc.dram_tensor("wl_t", [D, D], F32, kind="ExternalInput")
    wr_in = nc.dram_tensor("wr_t", [D, D], F32, kind="ExternalInput")
    bl_in = nc.dram_tensor("b_l", [D, 1], F32, kind="ExternalInput")
    out_ext = nc.dram_tensor("out_shard", [L * S, D], F32, kind="ExternalOutput")

    cc_in = nc.dram_tensor("cc_in", [S, D], F32, kind="Internal")
    cc_out = [
        nc.dram_tensor(f"cc_out{i}", [NPL, D], F32, kind="Internal", addr_space="Shared")
        for i in range(2)
    ]
    xt_dram = nc.dram_tensor("xt_dram", [D, L * S], F32, kind="Internal")
    groups = [list(range(M))]

    with tile.TileContext(nc) as tc:
        with (
            tc.tile_pool(name="const", bufs=1) as constp,
            tc.tile_pool(name="xtl", bufs=1) as xtlp,
            tc.tile_pool(name="xload", bufs=2) as xloadp,
            tc.tile_pool(name="gath", bufs=1) as gathp,
            tc.tile_pool(name="agg", bufs=2) as aggp,
            tc.tile_pool(name="sb", bufs=2) as sbp,
            tc.tile_pool(name="hts", bufs=2) as htp,
            tc.tile_pool(name="hnode", bufs=2) as hnp,
            tc.tile_pool(name="ptr", bufs=3, space="PSUM") as ptrp,
            tc.tile_pool(name="pu", bufs=2, space="PSUM") as pup,
            tc.tile_pool(name="phb", bufs=2, space="PSUM") as phbp,
        ):
            ident = constp.tile([128, 128], F32)
            make_identity(nc, ident[:])
            wl_sb = constp.tile([D, D], F32)
            nc.sync.dma_start(out=wl_sb[:], in_=wl_in[:])
            wr_sb = constp.tile([D, D], F32)
            nc.sync.dma_start(out=wr_sb[:], in_=wr_in[:])
            bl_sb = constp.tile([D, 1], F32)
            nc.sync.dma_start(out=bl_sb[:], in_=bl_in[:])
            idx_sb = constp.tile([128, (L - 1) * NPG], I16)
            nc.sync.dma_start(out=idx_sb[:], in_=idx_in[:])

            gdma_sem = nc.alloc_semaphore("gdma_sem")

            # gather buffers: double-buffered across levels (8 slots of
            # [128, 32, 64]) so next-level preps never wait on this level.
            gbufs = [
                gathp.tile([128, CIDX // 128, D], F32, tag=f"g{i}", name=f"gb{i}")
                for i in range(2 * NCHUNK)
            ]

            # ---- issue all gather preps + triggers (gpsimd stream) ----
            # Ring FIFO order = emission order. trigger(l) fires the 4 preps
            # of level l; its deferred RAW dep on cc_out[(l-1)%2] (written by
            # AllGather l-1) gates only the trigger, so descriptor generation
            # runs continuously ~a level ahead.
            use_prep = os.environ.get("DAGPROP_PREP", "0") == "1"

            # ---- preamble: load X shard, transpose to feature-major, spill ----
            for l in range(L):
                x_lvl = xloadp.tile([128, G * D], F32)
                nc.sync.dma_start(
                    out=x_lvl[:], in_=_nodemajor(x_in[l * S : (l + 1) * S, :])
                )
                xtbuf = sbp.tile([D, S], F32, tag="xtw", name=f"xtw{l}")
                for cc in range(4):  # 512-column chunks of xT
                    pt = ptrp.tile([64, 512], F32)
                    for j in range(4):
                        r = cc * 4 + j
                        nc.tensor.transpose(
                            out=pt[:, j * 128 : (j + 1) * 128],
                            in_=x_lvl[:, r * 64 : (r + 1) * 64],
                            identity=ident[:],
                        )
                    nc.vector.tensor_copy(
                        out=xtbuf[:, cc * 512 : (cc + 1) * 512], in_=pt[:]
                    )
                if l == 0:
                    xt0_sb = constp.tile([D, S], F32)
                    nc.vector.tensor_copy(out=xt0_sb[:], in_=xtbuf[:])
                else:
                    nc.sync.dma_start(
                        out=xt_dram[:, l * S : (l + 1) * S], in_=xtbuf[:]
                    )

            # ---- levels ----
            ag_insts = {}
            for l in range(L):
                hT = htp.tile([D, S], F32)
                if l == 0:
                    for cc in range(4):
                        pu = pup.tile([64, 512], F32)
                        nc.tensor.matmul(
                            out=pu[:],
                            lhsT=wr_sb[:],
                            rhs=xt0_sb[:, cc * 512 : (cc + 1) * 512],
                            start=True,
                            stop=True,
                        )
                        nc.vector.tensor_copy(
                            out=hT[:, cc * 512 : (cc + 1) * 512], in_=pu[:]
                        )
                else:
                    if use_prep:
                        src_cc = cc_out[(l - 1) % 2]
                        ibase = (l - 1) * NPG
                        for c in range(NCHUNK):
                            gb = gbufs[((l - 1) % 2) * NCHUNK + c]
                            prep = nc.gpsimd.dma_gather(
                                out_ap=gb[:],
                                in_ap=src_cc[:],
                                idxs_ap=idx_sb[:, ibase + c * (NPG // NCHUNK) : ibase + (c + 1) * (NPG // NCHUNK)],
                                num_idxs=CIDX,
                                num_idxs_reg=CIDX,
                                elem_size=D,
                                single_packet=False,
                                prepare_only=True,
                                sem=gdma_sem,
                            )
                            if l >= 2:
                                prep.ins.try_remove_dependency(ag_insts[l - 1].name)
                        trig = nc.gpsimd.trigger_dma(count=None)
                        add_dep_helper(
                            trig.ins,
                            ag_insts[l - 1],
                            sync=True,
                            reason="gather transfers wait for AllGather of prev level",
                        )
                    xt_lvl = xtlp.tile([D, S], F32)
                    nc.sync.dma_start(
                        out=xt_lvl[:], in_=xt_dram[:, l * S : (l + 1) * S]
                    )
                    for c in range(NCHUNK):
                        gb = gbufs[((l - 1) % 2) * NCHUNK + c]
                        if not use_prep:
                            ibase = (l - 1) * NPG
                            nc.gpsimd.dma_gather(
                                out_ap=gb[:],
                                in_ap=cc_out[(l - 1) % 2][:],
                                idxs_ap=idx_sb[:, ibase + c * (NPG // NCHUNK) : ibase + (c + 1) * (NPG // NCHUNK)],
                                num_idxs=CIDX,
                                num_idxs_reg=CIDX,
                                elem_size=D,
                                single_packet=False,
                            )
                        # reduce the 8 children of each parent (slot s*8+k)
                        gbr = gb[:].rearrange("p (s k) d -> p s d k", k=CH)
                        agg = aggp.tile([128, NCHUNK, D], F32)
                        nc.vector.tensor_reduce(
                            out=agg[:],
                            in_=gbr,
                            axis=mybir.AxisListType.X,
                            op=mybir.AluOpType.add,
                        )
                        agg2d = agg[:].rearrange("p s d -> p (s d)")
                        pt = ptrp.tile([64, 512], F32)
                        for j in range(4):
                            nc.tensor.transpose(
                                out=pt[:, j * 128 : (j + 1) * 128],
                                in_=agg2d[:, j * 64 : (j + 1) * 64],
                                identity=ident[:],
                            )
                        aggT = sbp.tile([64, 512], F32)
                        nc.vector.tensor_copy(out=aggT[:], in_=pt[:])
                        pu = pup.tile([64, 512], F32)
                        nc.tensor.matmul(
                            out=pu[:], lhsT=wl_sb[:], rhs=aggT[:], start=True, stop=False
                        )
                        nc.tensor.matmul(
                            out=pu[:],
                            lhsT=wr_sb[:],
                            rhs=xt_lvl[:, c * 512 : (c + 1) * 512],
                            start=False,
                            stop=True,
                        )
                        nc.scalar.activation(
                            out=hT[:, c * 512 : (c + 1) * 512],
                            in_=pu[:],
                            func=mybir.ActivationFunctionType.Tanh,
                            bias=bl_sb[:, 0:1],
                        )

                # transpose back to node-major and store
                h_node = hnp.tile([128, G * D], F32)
                for half in range(2):
                    phb = phbp.tile([128, 512], F32)
                    for j in range(8):
                        g = half * 8 + j
                        nc.tensor.transpose(
                            out=phb[:, j * 64 : (j + 1) * 64],
                            in_=hT[:, g * 128 : (g + 1) * 128],
                            identity=ident[:64, :64],
                        )
                    nc.vector.tensor_copy(
                        out=h_node[:, half * 512 : (half + 1) * 512], in_=phb[:]
                    )

                nc.sync.dma_start(
                    out=_nodemajor(out_ext[l * S : (l + 1) * S, :]), in_=h_node[:]
                )
                if l < L - 1:
                    nc.sync.dma_start(out=_nodemajor(cc_in[:, :]), in_=h_node[:])
                    ag = nc.gpsimd.collective_compute(
                        "AllGather",
                        mybir.AluOpType.bypass,
                        replica_groups=groups,
                        ins=[cc_in[:]],
                        outs=[cc_out[l % 2][:]],
                    )
                    ag_insts[l] = ag.ins

    nc.compile()
    return nc


def _prepare_inputs(x, edge_index, W_l, b_l, W_r):
    """Host-side prep: per-core shards and gather index tables."""
    src = np.asarray(edge_index[0])
    dst = np.asarray(edge_index[1])
    x = np.ascontiguousarray(np.asarray(x, dtype=np.float32))

    # children[l][j] = the 8 children (level-local ids) of node j of level l+1
    children = []
    for l in range(L - 1):
        lo, hi = l * NPL * CH, (l + 1) * NPL * CH
        d = dst[lo:hi]
        s = src[lo:hi]
        base = (l + 1) * NPL
        if not np.array_equal(d, np.repeat(np.arange(NPL, dtype=d.dtype) + base, CH)):
            order = np.argsort(d, kind="stable")
            d = d[order]
            s = s[order]
            assert np.array_equal(
                d, np.repeat(np.arange(NPL, dtype=d.dtype) + base, CH)
            ), "edge structure is not a layered DAG with 8 children per node"
        children.append((s.reshape(NPL, CH) - l * NPL).astype(np.int32))

    wl_t = np.ascontiguousarray(np.asarray(W_l, np.float32).T)
    wr_t = np.ascontiguousarray(np.asarray(W_r, np.float32).T)
    bl = np.ascontiguousarray(np.asarray(b_l, np.float32).reshape(D, 1))

    in_maps = []
    for c in range(M):
        xs = np.empty((L * S, D), np.float32)
        for l in range(L):
            xs[l * S : (l + 1) * S] = x[l * NPL + c * S : l * NPL + (c + 1) * S]
        idx = np.empty((128, (L - 1) * NPG), np.int16)
        for l in range(1, L):
            ch = children[l - 1][c * S : (c + 1) * S]  # [2048, 8]
            chv = ch.reshape(128, G, CH)  # [p, s, k]
            # gathered edge i lands at out[i % 128, i // 128]; we want edge
            # (parent p*16+s, child k) at slot s*8+k -> i = (s*8+k)*128 + p
            flat = chv.transpose(1, 2, 0).reshape(S * CH)
            w = flat.reshape(NPG, 16).T.astype(np.int16)  # wrapped: idx[q, t] = flat[t*16+q]
            for grp in range(8):
                idx[grp * 16 : (grp + 1) * 16, (l - 1) * NPG : l * NPG] = w
        in_maps.append(
            {
                "x_shard": xs,
                "idx": idx,
                "wl_t": wl_t,
                "wr_t": wr_t,
                "b_l": bl,
            }
        )
    return in_maps


_NC_CACHE = None


def kernel(x, edge_index, W_l, b_l, W_r, num_levels):
    global _NC_CACHE
    assert int(num_levels) == L
    assert x.shape == (L * NPL, D)

    in_maps = _prepare_inputs(x, edge_index, W_l, b_l, W_r)

    if _NC_CACHE is None:
        _NC_CACHE = _build_kernel()
    nc = _NC_CACHE

    trace = os.environ.get("DAGPROP_TRACE", "0") == "1"
    res = run_bass_kernel_spmd(
        nc,
        in_maps,
        core_ids=list(range(M)),
        trace=trace,
    )
    LAST_STATS["exec_time_ns"] = res.exec_time_ns
    LAST_STATS["mean_exec_time_ns"] = res.mean_exec_time_ns
    LAST_STATS["profile_json"] = res.profile_json
    LAST_STATS["instructions_and_trace"] = res.instructions_and_trace

    out = np.empty((L * NPL, D), np.float32)
    for c in range(M):
        o = res.results[c]["out_shard"]
        for l in range(L):
            out[l * NPL + c * S : l * NPL + (c + 1) * S] = o[l * S : (l + 1) * S]
    return out



# revision 2
# speedup vs baseline: 1.5124x; 1.5124x over previous
"""DAG propagation (layered GNN message passing) on 8 trn2 NeuronCores.

The reference reduces exactly (verified numerically) to:
    out[level 0] = X_0 @ W_r^T
    out[level l] = tanh(X_l @ W_r^T + (sum_k out[level l-1][child_k]) @ W_l^T + b_l)

Sharding: each core owns a contiguous 2048-node slice of every level.
Per level, each core dma_gathers its parents' 16384 children rows from a
replicated full h_prev (core-local DRAM), reduces the 8 children per
parent on the vector engine, computes its 2048 output rows, and an
AllGather replicates the new level for the next iteration.

The critical resource is the GpSimd Q7 SWDGE descriptor generation
(~8ns/gathered row, 114K rows/core, ~900us serial Q7 work — the floor of
this design). Gathers are chunked 4x per level so transfers/compute of
chunk c overlap descriptor generation of chunk c+1. (An experimental
prepare_only/trigger_dma mode that overlaps descgen with the AllGather
is behind DAGPROP_PREP=1; it is not stable on this runtime.)

Layouts: node n of a core's 2048-node shard lives at (partition, slot)
= (n // 16, n % 16) in node-major tiles (all HBM accesses per-partition
contiguous), and at column (n % 16)*128 + n // 16 in feature-major
[64, 2048] tiles, matching what PE transposes of slot-slices produce.
"""

import os
import numpy as np

import concourse.bass as bass
import concourse.bacc as bacc
import concourse.tile as tile
from concourse.tile import add_dep_helper
from concourse import mybir
from concourse.bass_utils import run_bass_kernel_spmd
from concourse.masks import make_identity

# Problem constants (hardcoded per spec; asserted against inputs at runtime).
L = 8            # levels
NPL = 16384      # nodes per level
CH = 8           # children per node
D = 64           # feature dim
M = 8            # cores
S = NPL // M     # 2048: shard nodes per level per core
G = S // 128     # 16: slots per partition in node-major tiles
NCHUNK = 4       # gather chunks per level
CIDX = S * CH // NCHUNK   # 4096 gathered rows per chunk
F32 = mybir.dt.float32
I16 = mybir.dt.int16
NPG = (S * CH) // 16      # 1024: idx columns per level (wrapped int16 layout)

LAST_STATS: dict = {}


def _nodemajor(ap):
    """View a [S, D] DRAM slice as [128, G*D] with row p*G+r on partition p."""
    return ap.rearrange("(p r) d -> p (r d)", p=128)


def _build_kernel():
    nc = bacc.Bacc(
        "TRN2",
        target_bir_lowering=False,
        debug=False,
        num_devices=M,
        dynamic_dma_scratch_size=32768,
        num_swdge_queues=4,
    )

    x_in = nc.dram_tensor("x_shard", [L * S, D], F32, kind="ExternalInput")
    idx_in = nc.dram_tensor("idx", [128, (L - 1) * NPG], I16, kind="ExternalInput")
    wl_in = nc.dram_tensor("wl_t", [D, D], F32, kind="ExternalInput")
    wr_in = nc.dram_tensor("wr_t", [D, D], F32, kind="ExternalInput")
    bl_in = nc.dram_tensor("b_l", [D, 1], F32, kind="ExternalInput")
    out_ext = nc.dram_tensor("out_shard", [L * S, D], F32, kind="ExternalOutput")

    cc_in = nc.dram_tensor("cc_in", [S, D], F32, kind="Internal")
    cc_out = [
        nc.dram_tensor(f"cc_out{i}", [NPL, D], F32, kind="Internal", addr_space="Shared")
        for i in range(2)
    ]
    xt_dram = nc.dram_tensor("xt_dram", [D, L * S], F32, kind="Internal")
    groups = [list(range(M))]

    with tile.TileContext(nc) as tc:
        with (
            tc.tile_pool(name="const", bufs=1) as constp,
            tc.tile_pool(name="xtl", bufs=1) as xtlp,
            tc.tile_pool(name="xload", bufs=2) as xloadp,
            tc.tile_pool(name="gath", bufs=1) as gathp,
            tc.tile_pool(name="agg", bufs=2) as aggp,
            tc.tile_pool(name="sb", bufs=2) as sbp,
            tc.tile_pool(name="hts", bufs=2) as htp,
            tc.tile_pool(name="hnode", bufs=2) as hnp,
            tc.tile_pool(name="ptr", bufs=3, space="PSUM") as ptrp,
            tc.tile_pool(name="pu", bufs=2, space="PSUM") as pup,
            tc.tile_pool(name="phb", bufs=2, space="PSUM") as phbp,
        ):
            ident = constp.tile([128, 128], F32)
            make_identity(nc, ident[:])
            wl_sb = constp.tile([D, D], F32)
            nc.sync.dma_start(out=wl_sb[:], in_=wl_in[:])
            wr_sb = constp.tile([D, D], F32)
            nc.sync.dma_start(out=wr_sb[:], in_=wr_in[:])
            bl_sb = constp.tile([D, 1], F32)
            nc.sync.dma_start(out=bl_sb[:], in_=bl_in[:])
            idx_sb = constp.tile([128, (L - 1) * NPG], I16)
            nc.sync.dma_start(out=idx_sb[:], in_=idx_in[:])

            gdma_sem = nc.alloc_semaphore("gdma_sem")

            # gather buffers: double-buffered across levels (8 slots of
            # [128, 32, 64]) so next-level preps never wait on this level.
            gbufs = [
                gathp.tile([128, CIDX // 128, D], F32, tag=f"g{i}", name=f"gb{i}")
                for i in range(2 * NCHUNK)
            ]

            # ---- issue all gather preps + triggers (gpsimd stream) ----
            # Ring FIFO order = emission order. trigger(l) fires the 4 preps
            # of level l; its deferred RAW dep on cc_out[(l-1)%2] (written by
            # AllGather l-1) gates only the trigger, so descriptor generation
            # runs continuously ~a level ahead.
            use_prep = os.environ.get("DAGPROP_PREP", "0") == "1"

            # ---- preamble: load X shard, transpose to feature-major, spill ----
            for l in range(L):
                x_lvl = xloadp.tile([128, G * D], F32)
                nc.sync.dma_start(
                    out=x_lvl[:], in_=_nodemajor(x_in[l * S : (l + 1) * S, :])
                )
                xtbuf = sbp.tile([D, S], F32, tag="xtw", name=f"xtw{l}")
                for cc in range(4):  # 512-column chunks of xT
                    pt = ptrp.tile([64, 512], F32)
                    for j in range(4):
                        r = cc * 4 + j
                        nc.tensor.transpose(
                            out=pt[:, j * 128 : (j + 1) * 128],
                            in_=x_lvl[:, r * 64 : (r + 1) * 64],
                            identity=ident[:],
                        )
                    nc.vector.tensor_copy(
                        out=xtbuf[:, cc * 512 : (cc + 1) * 512], in_=pt[:]
                    )
                if l == 0:
                    xt0_sb = constp.tile([D, S], F32)
                    nc.vector.tensor_copy(out=xt0_sb[:], in_=xtbuf[:])
                else:
                    nc.sync.dma_start(
                        out=xt_dram[:, l * S : (l + 1) * S], in_=xtbuf[:]
                    )

            # ---- levels ----
            ag_insts = {}
            for l in range(L):
                hT = htp.tile([D, S], F32)
                if l == 0:
                    for cc in range(4):
                        pu = pup.tile([64, 512], F32)
                        nc.tensor.matmul(
                            out=pu[:],
                            lhsT=wr_sb[:],
                            rhs=xt0_sb[:, cc * 512 : (cc + 1) * 512],
                            start=True,
                            stop=True,
                        )
                        nc.vector.tensor_copy(
                            out=hT[:, cc * 512 : (cc + 1) * 512], in_=pu[:]
                        )
                else:
                    if use_prep:
                        src_cc = cc_out[(l - 1) % 2]
                        ibase = (l - 1) * NPG
                        for c in range(NCHUNK):
                            gb = gbufs[((l - 1) % 2) * NCHUNK + c]
                            prep = nc.gpsimd.dma_gather(
                                out_ap=gb[:],
                                in_ap=src_cc[:],
                                idxs_ap=idx_sb[:, ibase + c * (NPG // NCHUNK) : ibase + (c + 1) * (NPG // NCHUNK)],
                                num_idxs=CIDX,
                                num_idxs_reg=CIDX,
                                elem_size=D,
                                single_packet=False,
                                prepare_only=True,
                                sem=gdma_sem,
                            )
                            if l >= 2:
                                prep.ins.try_remove_dependency(ag_insts[l - 1].name)
                        trig = nc.gpsimd.trigger_dma(count=None)
                        add_dep_helper(
                            trig.ins,
                            ag_insts[l - 1],
                            sync=True,
                            reason="gather transfers wait for AllGather of prev level",
                        )
                    xt_lvl = xtlp.tile([D, S], F32)
                    nc.sync.dma_start(
                        out=xt_lvl[:], in_=xt_dram[:, l * S : (l + 1) * S]
                    )
                    for c in range(NCHUNK):
                        gb = gbufs[((l - 1) % 2) * NCHUNK + c]
                        if not use_prep:
                            ibase = (l - 1) * NPG
                            nc.gpsimd.dma_gather(
                                out_ap=gb[:],
                                in_ap=cc_out[(l - 1) % 2][:],
                                idxs_ap=idx_sb[:, ibase + c * (NPG // NCHUNK) : ibase + (c + 1) * (NPG // NCHUNK)],
                                num_idxs=CIDX,
                                num_idxs_reg=CIDX,
                                elem_size=D,
                                single_packet=False,
                                queue_num=c,
                            )
                        # reduce the 8 children of each parent (slot s*8+k)
                        gbr = gb[:].rearrange("p (s k) d -> p s d k", k=CH)
                        agg = aggp.tile([128, NCHUNK, D], F32)
                        nc.vector.tensor_reduce(
                            out=agg[:],
                            in_=gbr,
                            axis=mybir.AxisListType.X,
                            op=mybir.AluOpType.add,
                        )
                        agg2d = agg[:].rearrange("p s d -> p (s d)")
                        pt = ptrp.tile([64, 512], F32)
                        for j in range(4):
                            nc.tensor.transpose(
                                out=pt[:, j * 128 : (j + 1) * 128],
                                in_=agg2d[:, j * 64 : (j + 1) * 64],
                                identity=ident[:],
                            )
                        aggT = sbp.tile([64, 512], F32)
                        nc.vector.tensor_copy(out=aggT[:], in_=pt[:])
                        pu = pup.tile([64, 512], F32)
                        nc.tensor.matmul(
                            out=pu[:], lhsT=wl_sb[:], rhs=aggT[:], start=True, stop=False
                        )
                        nc.tensor.matmul(
                            out=pu[:],
                            lhsT=wr_sb[:],
                            rhs=xt_lvl[:, c * 512 : (c + 1) * 512],
                            start=False,
                            stop=True,
                        )
                        nc.scalar.activation(
                            out=hT[:, c * 512 : (c + 1) * 512],
                            in_=pu[:],
                            func=mybir.ActivationFunctionType.Tanh,
                            bias=bl_sb[:, 0:1],
                        )

                # transpose back to node-major and store
                h_node = hnp.tile([128, G * D], F32)
                for half in range(2):
                    phb = phbp.tile([128, 512], F32)
                    for j in range(8):
                        g = half * 8 + j
                        nc.tensor.transpose(
                            out=phb[:, j * 64 : (j + 1) * 64],
                            in_=hT[:, g * 128 : (g + 1) * 128],
                            identity=ident[:64, :64],
                        )
                    nc.vector.tensor_copy(
                        out=h_node[:, half * 512 : (half + 1) * 512], in_=phb[:]
                    )

                nc.sync.dma_start(
                    out=_nodemajor(out_ext[l * S : (l + 1) * S, :]), in_=h_node[:]
                )
                if l < L - 1:
                    nc.sync.dma_start(out=_nodemajor(cc_in[:, :]), in_=h_node[:])
                    ag = nc.gpsimd.collective_compute(
                        "AllGather",
                        mybir.AluOpType.bypass,
                        replica_groups=groups,
                        ins=[cc_in[:]],
                        outs=[cc_out[l % 2][:]],
                    )
                    ag_insts[l] = ag.ins

    nc.compile()
    return nc


def _prepare_inputs(x, edge_index, W_l, b_l, W_r):
    """Host-side prep: per-core shards and gather index tables."""
    src = np.asarray(edge_index[0])
    dst = np.asarray(edge_index[1])
    x = np.ascontiguousarray(np.asarray(x, dtype=np.float32))

    # children[l][j] = the 8 children (level-local ids) of node j of level l+1
    children = []
    for l in range(L - 1):
        lo, hi = l * NPL * CH, (l + 1) * NPL * CH
        d = dst[lo:hi]
        s = src[lo:hi]
        base = (l + 1) * NPL
        if not np.array_equal(d, np.repeat(np.arange(NPL, dtype=d.dtype) + base, CH)):
            order = np.argsort(d, kind="stable")
            d = d[order]
            s = s[order]
            assert np.array_equal(
                d, np.repeat(np.arange(NPL, dtype=d.dtype) + base, CH)
            ), "edge structure is not a layered DAG with 8 children per node"
        children.append((s.reshape(NPL, CH) - l * NPL).astype(np.int32))

    wl_t = np.ascontiguousarray(np.asarray(W_l, np.float32).T)
    wr_t = np.ascontiguousarray(np.asarray(W_r, np.float32).T)
    bl = np.ascontiguousarray(np.asarray(b_l, np.float32).reshape(D, 1))

    in_maps = []
    for c in range(M):
        xs = np.empty((L * S, D), np.float32)
        for l in range(L):
            xs[l * S : (l + 1) * S] = x[l * NPL + c * S : l * NPL + (c + 1) * S]
        idx = np.empty((128, (L - 1) * NPG), np.int16)
        for l in range(1, L):
            ch = children[l - 1][c * S : (c + 1) * S]  # [2048, 8]
            chv = ch.reshape(128, G, CH)  # [p, s, k]
            # gathered edge i lands at out[i % 128, i // 128]; we want edge
            # (parent p*16+s, child k) at slot s*8+k -> i = (s*8+k)*128 + p
            flat = chv.transpose(1, 2, 0).reshape(S * CH)
            w = flat.reshape(NPG, 16).T.astype(np.int16)  # wrapped: idx[q, t] = flat[t*16+q]
            for grp in range(8):
                idx[grp * 16 : (grp + 1) * 16, (l - 1) * NPG : l * NPG] = w
        in_maps.append(
            {
                "x_shard": xs,
                "idx": idx,
                "wl_t": wl_t,
                "wr_t": wr_t,
                "b_l": bl,
            }
        )
    return in_maps


_NC_CACHE = None


def kernel(x, edge_index, W_l, b_l, W_r, num_levels):
    global _NC_CACHE
    assert int(num_levels) == L
    assert x.shape == (L * NPL, D)

    in_maps = _prepare_inputs(x, edge_index, W_l, b_l, W_r)

    if _NC_CACHE is None:
        _NC_CACHE = _build_kernel()
    nc = _NC_CACHE

    trace = os.environ.get("DAGPROP_TRACE", "0") == "1"
    res = run_bass_kernel_spmd(
        nc,
        in_maps,
        core_ids=list(range(M)),
        trace=trace,
    )
    LAST_STATS["exec_time_ns"] = res.exec_time_ns
    LAST_STATS["mean_exec_time_ns"] = res.mean_exec_time_ns
    LAST_STATS["profile_json"] = res.profile_json
    LAST_STATS["instructions_and_trace"] = res.instructions_and_trace

    out = np.empty((L * NPL, D), np.float32)
    for c in range(M):
        o = res.results[c]["out_shard"]
        for l in range(L):
            out[l * NPL + c * S : l * NPL + (c + 1) * S] = o[l * S : (l + 1) * S]
    return out



# revision 8
# speedup vs baseline: 1.8341x; 1.2127x over previous
"""DAG propagation (layered GNN message passing) on 8 trn2 NeuronCores.

The reference reduces exactly (verified numerically) to:
    out[level 0] = X_0 @ W_r^T
    out[level l] = tanh(X_l @ W_r^T + (sum_k out[level l-1][child_k]) @ W_l^T + b_l)

Sharding: each core owns a contiguous 2048-node slice of every level.
Per level, each core dma_gathers its parents' 16384 children rows from a
replicated full h_prev (core-local DRAM), reduces the 8 children per
parent on the vector engine, computes its 2048 output rows, and an
AllGather replicates the new level for the next iteration.

The critical resource is the GpSimd Q7 SWDGE descriptor generation
(~8ns/gathered row, 114K rows/core, ~900us serial Q7 work — the floor of
this design). Gathers are chunked 4x per level so transfers/compute of
chunk c overlap descriptor generation of chunk c+1. (An experimental
prepare_only/trigger_dma mode that overlaps descgen with the AllGather
is behind DAGPROP_PREP=1; it is not stable on this runtime.)

Layouts: node n of a core's 2048-node shard lives at (partition, slot)
= (n // 16, n % 16) in node-major tiles (all HBM accesses per-partition
contiguous), and at column (n % 16)*128 + n // 16 in feature-major
[64, 2048] tiles, matching what PE transposes of slot-slices produce.
"""

import os
import numpy as np

import concourse.bass as bass
import concourse.bacc as bacc
import concourse.tile as tile
from concourse.tile import add_dep_helper
from concourse import mybir
from concourse.bass_utils import run_bass_kernel_spmd
from concourse.masks import make_identity

# Problem constants (hardcoded per spec; asserted against inputs at runtime).
L = 8            # levels
NPL = 16384      # nodes per level
CH = 8           # children per node
D = 64           # feature dim
M = 8            # cores
S = NPL // M     # 2048: shard nodes per level per core
G = S // 128     # 16: slots per partition in node-major tiles
NCHUNK = 8       # gather chunks per level (2 rounds over 4 SWDGE queues)
CIDX = S * CH // NCHUNK   # 2048 gathered rows per chunk
CW = S // NCHUNK          # 256: hT columns per chunk
NT = CW // 128            # 2: PE transposes per chunk
F32 = mybir.dt.float32
I16 = mybir.dt.int16
NPG = (S * CH) // 16      # 1024: idx columns per level (wrapped int16 layout)

LAST_STATS: dict = {}


def _nodemajor(ap):
    """View a [S, D] DRAM slice as [128, G*D] with row p*G+r on partition p."""
    return ap.rearrange("(p r) d -> p (r d)", p=128)


def _build_kernel():
    nc = bacc.Bacc(
        "TRN2",
        target_bir_lowering=False,
        debug=False,
        num_devices=M,
        dynamic_dma_scratch_size=32768,
        num_swdge_queues=4,
    )

    x_in = nc.dram_tensor("x_shard", [L * S, D], F32, kind="ExternalInput")
    idx_in = nc.dram_tensor("idx", [128, (L - 1) * NPG], I16, kind="ExternalInput")
    wl_in = nc.dram_tensor("wl_t", [D, D], F32, kind="ExternalInput")
    wr_in = nc.dram_tensor("wr_t", [D, D], F32, kind="ExternalInput")
    bl_in = nc.dram_tensor("b_l", [D, 1], F32, kind="ExternalInput")
    out_ext = nc.dram_tensor("out_shard", [L * S, D], F32, kind="ExternalOutput")

    cc_in = nc.dram_tensor("cc_in", [S, D], F32, kind="Internal")
    cc_out = [
        nc.dram_tensor(f"cc_out{i}", [NPL, D], F32, kind="Internal", addr_space="Shared")
        for i in range(2)
    ]
    xt_dram = nc.dram_tensor("xt_dram", [D, L * S], F32, kind="Internal")
    groups = [list(range(M))]

    with tile.TileContext(nc) as tc:
        with (
            tc.tile_pool(name="const", bufs=1) as constp,
            tc.tile_pool(name="xtl", bufs=1) as xtlp,
            tc.tile_pool(name="xload", bufs=2) as xloadp,
            tc.tile_pool(name="gath", bufs=1) as gathp,
            tc.tile_pool(name="agg", bufs=2) as aggp,
            tc.tile_pool(name="sb", bufs=2) as sbp,
            tc.tile_pool(name="hts", bufs=2) as htp,
            tc.tile_pool(name="hnode", bufs=2) as hnp,
            tc.tile_pool(name="ptr", bufs=3, space="PSUM") as ptrp,
            tc.tile_pool(name="pu", bufs=2, space="PSUM") as pup,
            tc.tile_pool(name="phb", bufs=2, space="PSUM") as phbp,
        ):
            ident = constp.tile([128, 128], F32)
            make_identity(nc, ident[:])
            wl_sb = constp.tile([D, D], F32)
            nc.sync.dma_start(out=wl_sb[:], in_=wl_in[:])
            wr_sb = constp.tile([D, D], F32)
            nc.sync.dma_start(out=wr_sb[:], in_=wr_in[:])
            bl_sb = constp.tile([D, 1], F32)
            nc.sync.dma_start(out=bl_sb[:], in_=bl_in[:])
            idx_sb = constp.tile([128, (L - 1) * NPG], I16)
            nc.sync.dma_start(out=idx_sb[:], in_=idx_in[:])

            gdma_sem = nc.alloc_semaphore("gdma_sem")

            # gather buffers: double-buffered across levels (8 slots of
            # [128, 32, 64]) so next-level preps never wait on this level.
            gbufs = [
                gathp.tile([128, CIDX // 128, D], F32, tag=f"g{i}", name=f"gb{i}")
                for i in range(2 * NCHUNK)
            ]

            # ---- issue all gather preps + triggers (gpsimd stream) ----
            # Ring FIFO order = emission order. trigger(l) fires the 4 preps
            # of level l; its deferred RAW dep on cc_out[(l-1)%2] (written by
            # AllGather l-1) gates only the trigger, so descriptor generation
            # runs continuously ~a level ahead.
            use_prep = os.environ.get("DAGPROP_PREP", "0") == "1"

            # ---- X preamble: load, transpose to feature-major, keep/spill.
            # Level 0's slice is emitted BEFORE the level loop; levels 1-7
            # are emitted after AllGather(0) is issued so they overlap it.
            def emit_x_level(l):
                x_lvl = xloadp.tile([128, G * D], F32)
                nc.sync.dma_start(
                    out=x_lvl[:], in_=_nodemajor(x_in[l * S : (l + 1) * S, :])
                )
                xtbuf = sbp.tile([D, S], F32, tag="xtw", name=f"xtw{l}")
                for cc in range(4):  # 512-column chunks of xT
                    pt = ptrp.tile([64, 512], F32)
                    for j in range(4):
                        r = cc * 4 + j
                        nc.tensor.transpose(
                            out=pt[:, j * 128 : (j + 1) * 128],
                            in_=x_lvl[:, r * 64 : (r + 1) * 64],
                            identity=ident[:],
                        )
                    nc.vector.tensor_copy(
                        out=xtbuf[:, cc * 512 : (cc + 1) * 512], in_=pt[:]
                    )
                if l == 0:
                    nc.vector.tensor_copy(out=xt0_sb[:], in_=xtbuf[:])
                else:
                    nc.sync.dma_start(
                        out=xt_dram[:, l * S : (l + 1) * S], in_=xtbuf[:]
                    )

            xt0_sb = constp.tile([D, S], F32)
            emit_x_level(0)

            # ---- levels ----
            ag_insts = {}
            for l in range(L):
                hT = htp.tile([D, S], F32)
                if l == 0:
                    for cc in range(4):
                        pu = pup.tile([64, 512], F32)
                        nc.tensor.matmul(
                            out=pu[:],
                            lhsT=wr_sb[:],
                            rhs=xt0_sb[:, cc * 512 : (cc + 1) * 512],
                            start=True,
                            stop=True,
                        )
                        nc.vector.tensor_copy(
                            out=hT[:, cc * 512 : (cc + 1) * 512], in_=pu[:]
                        )
                else:
                    if use_prep:
                        src_cc = cc_out[(l - 1) % 2]
                        ibase = (l - 1) * NPG
                        for c in range(NCHUNK):
                            gb = gbufs[((l - 1) % 2) * NCHUNK + c]
                            prep = nc.gpsimd.dma_gather(
                                out_ap=gb[:],
                                in_ap=src_cc[:],
                                idxs_ap=idx_sb[:, ibase + c * (NPG // NCHUNK) : ibase + (c + 1) * (NPG // NCHUNK)],
                                num_idxs=CIDX,
                                num_idxs_reg=CIDX,
                                elem_size=D,
                                single_packet=False,
                                prepare_only=True,
                                sem=gdma_sem,
                            )
                            if l >= 2:
                                prep.ins.try_remove_dependency(ag_insts[l - 1].name)
                        trig = nc.gpsimd.trigger_dma(count=None)
                        add_dep_helper(
                            trig.ins,
                            ag_insts[l - 1],
                            sync=True,
                            reason="gather transfers wait for AllGather of prev level",
                        )
                    xt_lvl = xtlp.tile([D, S], F32)
                    nc.sync.dma_start(
                        out=xt_lvl[:], in_=xt_dram[:, l * S : (l + 1) * S]
                    )
                    for c in range(NCHUNK):
                        gb = gbufs[((l - 1) % 2) * NCHUNK + c]
                        if not use_prep:
                            ibase = (l - 1) * NPG
                            nc.gpsimd.dma_gather(
                                out_ap=gb[:],
                                in_ap=cc_out[(l - 1) % 2][:],
                                idxs_ap=idx_sb[:, ibase + c * (NPG // NCHUNK) : ibase + (c + 1) * (NPG // NCHUNK)],
                                num_idxs=CIDX,
                                num_idxs_reg=CIDX,
                                elem_size=D,
                                single_packet=False,
                                queue_num=c % 4,
                            )
                        # reduce the 8 children of each parent (slot s*8+k)
                        gbr = gb[:].rearrange("p (s k) d -> p s d k", k=CH)
                        agg = aggp.tile([128, CIDX // 128 // CH, D], F32)
                        nc.vector.tensor_reduce(
                            out=agg[:],
                            in_=gbr,
                            axis=mybir.AxisListType.X,
                            op=mybir.AluOpType.add,
                        )
                        agg2d = agg[:].rearrange("p s d -> p (s d)")
                        pt = ptrp.tile([64, CW], F32)
                        for j in range(NT):
                            nc.tensor.transpose(
                                out=pt[:, j * 128 : (j + 1) * 128],
                                in_=agg2d[:, j * 64 : (j + 1) * 64],
                                identity=ident[:],
                            )
                        aggT = sbp.tile([64, CW], F32)
                        nc.vector.tensor_copy(out=aggT[:], in_=pt[:])
                        pu = pup.tile([64, CW], F32)
                        nc.tensor.matmul(
                            out=pu[:], lhsT=wl_sb[:], rhs=aggT[:], start=True, stop=False
                        )
                        nc.tensor.matmul(
                            out=pu[:],
                            lhsT=wr_sb[:],
                            rhs=xt_lvl[:, c * CW : (c + 1) * CW],
                            start=False,
                            stop=True,
                        )
                        nc.scalar.activation(
                            out=hT[:, c * CW : (c + 1) * CW],
                            in_=pu[:],
                            func=mybir.ActivationFunctionType.Tanh,
                            bias=bl_sb[:, 0:1],
                        )

                # transpose back to node-major and store
                h_node = hnp.tile([128, G * D], F32)
                for half in range(2):
                    phb = phbp.tile([128, 512], F32)
                    for j in range(8):
                        g = half * 8 + j
                        nc.tensor.transpose(
                            out=phb[:, j * 64 : (j + 1) * 64],
                            in_=hT[:, g * 128 : (g + 1) * 128],
                            identity=ident[:64, :64],
                        )
                    nc.vector.tensor_copy(
                        out=h_node[:, half * 512 : (half + 1) * 512], in_=phb[:]
                    )

                nc.sync.dma_start(
                    out=_nodemajor(out_ext[l * S : (l + 1) * S, :]), in_=h_node[:]
                )
                if l < L - 1:
                    nc.sync.dma_start(out=_nodemajor(cc_in[:, :]), in_=h_node[:])
                    ag = nc.gpsimd.collective_compute(
                        "AllGather",
                        mybir.AluOpType.bypass,
                        replica_groups=groups,
                        ins=[cc_in[:]],
                        outs=[cc_out[l % 2][:]],
                    )
                    ag_insts[l] = ag.ins
                if l == 0:
                    # rest of the X preamble overlaps AllGather(0) + levels
                    for ll in range(1, L):
                        emit_x_level(ll)

    nc.compile()
    return nc


def _prepare_inputs(x, edge_index, W_l, b_l, W_r):
    """Host-side prep: per-core shards and gather index tables."""
    src = np.asarray(edge_index[0])
    dst = np.asarray(edge_index[1])
    x = np.ascontiguousarray(np.asarray(x, dtype=np.float32))

    # children[l][j] = the 8 children (level-local ids) of node j of level l+1
    children = []
    for l in range(L - 1):
        lo, hi = l * NPL * CH, (l + 1) * NPL * CH
        d = dst[lo:hi]
        s = src[lo:hi]
        base = (l + 1) * NPL
        if not np.array_equal(d, np.repeat(np.arange(NPL, dtype=d.dtype) + base, CH)):
            order = np.argsort(d, kind="stable")
            d = d[order]
            s = s[order]
            assert np.array_equal(
                d, np.repeat(np.arange(NPL, dtype=d.dtype) + base, CH)
            ), "edge structure is not a layered DAG with 8 children per node"
        children.append((s.reshape(NPL, CH) - l * NPL).astype(np.int32))

    wl_t = np.ascontiguousarray(np.asarray(W_l, np.float32).T)
    wr_t = np.ascontiguousarray(np.asarray(W_r, np.float32).T)
    bl = np.ascontiguousarray(np.asarray(b_l, np.float32).reshape(D, 1))

    in_maps = []
    for c in range(M):
        xs = np.empty((L * S, D), np.float32)
        for l in range(L):
            xs[l * S : (l + 1) * S] = x[l * NPL + c * S : l * NPL + (c + 1) * S]
        idx = np.empty((128, (L - 1) * NPG), np.int16)
        for l in range(1, L):
            ch = children[l - 1][c * S : (c + 1) * S]  # [2048, 8]
            chv = ch.reshape(128, G, CH)  # [p, s, k]
            # gathered edge i lands at out[i % 128, i // 128]; we want edge
            # (parent p*16+s, child k) at slot s*8+k -> i = (s*8+k)*128 + p
            flat = chv.transpose(1, 2, 0).reshape(S * CH)
            w = flat.reshape(NPG, 16).T.astype(np.int16)  # wrapped: idx[q, t] = flat[t*16+q]
            for grp in range(8):
                idx[grp * 16 : (grp + 1) * 16, (l - 1) * NPG : l * NPG] = w
        in_maps.append(
            {
                "x_shard": xs,
                "idx": idx,
                "wl_t": wl_t,
                "wr_t": wr_t,
                "b_l": bl,
            }
        )
    return in_maps


_NC_CACHE = None


def kernel(x, edge_index, W_l, b_l, W_r, num_levels):
    global _NC_CACHE
    assert int(num_levels) == L
    assert x.shape == (L * NPL, D)

    in_maps = _prepare_inputs(x, edge_index, W_l, b_l, W_r)

    if _NC_CACHE is None:
        _NC_CACHE = _build_kernel()
    nc = _NC_CACHE

    trace = os.environ.get("DAGPROP_TRACE", "0") == "1"
    res = run_bass_kernel_spmd(
        nc,
        in_maps,
        core_ids=list(range(M)),
        trace=trace,
    )
    LAST_STATS["exec_time_ns"] = res.exec_time_ns
    LAST_STATS["mean_exec_time_ns"] = res.mean_exec_time_ns
    LAST_STATS["profile_json"] = res.profile_json
    LAST_STATS["instructions_and_trace"] = res.instructions_and_trace

    out = np.empty((L * NPL, D), np.float32)
    for c in range(M):
        o = res.results[c]["out_shard"]
        for l in range(L):
            out[l * NPL + c * S : l * NPL + (c + 1) * S] = o[l * S : (l + 1) * S]
    return out

